# revision 20
# baseline (speedup 1.0000x reference)
"""DBRX router kernel for Trainium2 (8 NeuronCores, data-parallel).

Problem: x [4, 4096, 6144] f32, W [6144, 16] f32
  logits  = x @ W                      (contraction over d=6144)
  weights = softmax(logits, axis=-1)
  top_w, top_e = top_k(weights, 4);  top_w /= sum(top_w)

Design (per core, 2048 tokens):
  * x is loaded with a strided DMA that places (d-block g, token q) on the 128
    partitions and (token-block c, d-offset f) on the free dim.  A single DVE
    stream_transpose (32x32 block transpose) then yields tiles where partition
    p holds d = 1536*(p//32) + 32*k + (p%32) and the free dim is tokens.  The
    scrambled d-order is absorbed by pre-permuting W on the host, since the
    matmul contraction is permutation invariant.
  * 48 accumulating K=128 / N=256 fp32 matmuls per 256-token group produce
    logits^T in PSUM.  fp32 streams at 1/4 rate but M=16 uses only 16 of the
    PE's 128 columns, so the matmuls are issued 4-way column-tiled
    (tile_position=(0, 32g)): four k-chunks stream concurrently in different
    32-column strips, recovering the 4x.  A single K=128 select-matmul folds
    the four partial strips into logits^T [16, 256].  (float32r would give
    the same rate in one stream, but its ~1e-4 product rounding flips ~20
    of 65536 top-4 expert selections — not exact enough for an absmax-style
    gate, so exact fp32 it is.)
  * Tiny logits are PE-transposed back to [128 tokens, 16 experts]; softmax is
    exp (ACT, fused row-sum) * reciprocal (DVE); top-4 comes from the DVE
    max8/max_index instructions; top weights are L1-renormalized.
  * All outputs are staged transposed in SBUF so output DMAs are contiguous;
    the host undoes the transposes (cheap: outputs are ~1.5 MB total).
"""

import sys

for _p in ("/opt/trn_rl_repo",):
    if _p not in sys.path:
        sys.path.append(_p)

from contextlib import ExitStack

import numpy as np

import concourse.bacc as bacc
import concourse.bass as bass
import concourse.mybir as mybir
import concourse.tile as tile
from concourse import bass_utils

# ---- problem constants (hardcoded per contest contract) ----
B, S, D, E = 4, 4096, 6144, 16
TOPK = 4
NCORES = 8
TOK = (B * S) // NCORES          # 2048 tokens per core
NST = TOK // 128                 # 16 supertiles (128 tokens each)
GROUPS = NST // 2                # 8 groups of 256 tokens
KB = 48                          # matmul K-steps per group (48*128 = 6144)
GSPAN = D // 4                   # 1536: d-span per partition-block

F32 = mybir.dt.float32
F32R = mybir.dt.float32r
U16 = mybir.dt.uint16
F16 = mybir.dt.float16


def _router_body(ctx, tc, outs, ins):
    nc = tc.nc
    xs, w128, eye, eyeu = ins["xs"], ins["w128"], ins["eye"], ins["eyeu"]
    wT, twT, teT = outs["wT"], outs["twT"], outs["teT"]
    Exp = mybir.ActivationFunctionType.Exp
    Copy = mybir.ActivationFunctionType.Copy

    const = ctx.enter_context(tc.tile_pool(name="const", bufs=1))
    xn_pool = ctx.enter_context(tc.tile_pool(name="xn", bufs=2))
    xt_pool = ctx.enter_context(tc.tile_pool(name="xt", bufs=2))
    stage = ctx.enter_context(tc.tile_pool(name="stage", bufs=1))
    small = ctx.enter_context(tc.tile_pool(name="small", bufs=3))
    ps_pool = ctx.enter_context(tc.tile_pool(name="ps", bufs=2, space="PSUM"))
    lp_pool = ctx.enter_context(tc.tile_pool(name="lp", bufs=2, space="PSUM"))
    tp_pool = ctx.enter_context(tc.tile_pool(name="tp", bufs=2, space="PSUM"))

    w128_r = const.tile([128, KB * E], F32)
    nc.sync.dma_start(w128_r[:], w128[:])
    eye_sb = const.tile([128, 128], F32)
    nc.sync.dma_start(eye_sb[:], eye[:])
    eyeu_sb = const.tile([128, 128], F16)
    nc.sync.dma_start(eyeu_sb[:], eyeu[:])
    sum4_sb = const.tile([128, E], F32)
    nc.sync.dma_start(sum4_sb[:], ins["sum4"][:])

    lT = stage.tile([16, TOK], F32)        # logits^T staging
    # Strip-evacuation staging: partitions [32g, 32g+16) carry strip g's
    # partial logits; the gap rows stay zero (memset once) so the fold-matmul
    # can read all 128 partitions (sum4 has zero rows there anyway).
    ps_sb = stage.tile([128, 256], F32)
    nc.vector.memset(ps_sb[:], 0.0)
    wT_stage = stage.tile([16, TOK], F32)  # softmax weights^T staging
    tw_stage = stage.tile([128, NST * TOPK], F32)
    te_stage = stage.tile([128, NST * TOPK], F16)

    # DRAM view: (st, g) -> [32, 4, 1536] element stream in (q, c, f) order,
    # i.e. xs[128*st + 32*c + q, 1536*g + f]; this matches the SBUF-side
    # partition-slice [32 partitions = q, free = (c, f)] tile element order.
    # (One DMA per (supertile, partition-block): the DMA AP balancer caps
    # either side at 3 dims.)
    xsv = xs.rearrange("(st c q) (g f) -> st g q c f", c=4, q=32, g=4)

    for gi in range(GROUPS):
        # ---- load + on-chip transpose of 256 tokens ----
        xt = xt_pool.tile([128, 2 * D], F32)
        for h in range(2):
            st = gi * 2 + h
            xn = xn_pool.tile([128, D], F32)
            for g in range(4):
                nc.sync.dma_start(xn[32 * g:32 * (g + 1), :], xsv[st, g])
            # After this, xt[p, 1536*cc + 32*k + j] (cc = 4*h + c) holds
            # x[token(st, c, j), 1536*(p//32) + 32*k + (p%32)].
            nc.vector.transpose(xt[:, h * D:(h + 1) * D], xn[:])

        # ---- 48 accumulating matmuls: logits^T, 4-way column-tiled ----
        # fp32 matmuls stream at 1/4 rate, but M=16 uses only 16 of the PE
        # array's 128 columns.  Column tiling runs 4 k-chunks concurrently in
        # different 32-column strips; strip g accumulates chunks k === g (mod
        # 4) into PSUM partitions [32g, 32g+16).
        ps = ps_pool.tile([128, 256], F32)
        xtr = xt[:].rearrange("p (cc k j) -> p k cc j", cc=8, k=KB, j=32)
        for k in range(KB):
            g = k % 4
            nc.tensor.matmul(
                ps[32 * g:32 * g + E, :],
                lhsT=w128_r[:, k * E:(k + 1) * E],
                rhs=xtr[:, k],
                start=(k == g),
                stop=(k == KB - 4 + g),
                tile_position=(0, 32 * g),
                # The 4 strips keep independent per-partition accumulation
                # groups inside one PSUM bank; CoreSim's zero-region race
                # bookkeeping can't represent that (numerics are unaffected).
                skip_group_check=True,
            )
        # Evacuate the 4 partial strips, then fold them with one K=128
        # select-matmul: sum4sel[p, e] = 1 iff p % 32 == e.
        for g in range(4):
            nc.scalar.activation(ps_sb[32 * g:32 * g + E, :],
                                 ps[32 * g:32 * g + E, :], Copy)
        lps = lp_pool.tile([16, 256], F32, tag="lps")
        nc.tensor.matmul(lps[:], lhsT=sum4_sb[:], rhs=ps_sb[:], start=True,
                         stop=True)
        nc.scalar.activation(lT[:, gi * 256:(gi + 1) * 256], lps[:], Copy)

        # ---- per-128-token epilogue ----
        for h in range(2):
            t = gi * 2 + h
            lp = lp_pool.tile([128, E], F32)
            nc.tensor.transpose(lp[:], lT[:, t * 128:(t + 1) * 128],
                                eye_sb[0:16, 0:16])
            e_sb = small.tile([128, E], F32, tag="esb")
            s1 = small.tile([128, 1], F32, tag="s1")
            nc.scalar.activation(e_sb[:], lp[:], Exp, accum_out=s1[:])
            r1 = small.tile([128, 1], F32, tag="r1")
            nc.vector.reciprocal(r1[:], s1[:])
            w_sb = small.tile([128, E], F32, tag="wsb")
            nc.vector.tensor_scalar_mul(w_sb[:], e_sb[:], r1[:])
            wtp = tp_pool.tile([16, 128], F32, tag="wtp")
            nc.tensor.transpose(wtp[:], w_sb[:], eye_sb[:])
            nc.scalar.activation(wT_stage[:, t * 128:(t + 1) * 128], wtp[:],
                                 Copy)
            m8 = small.tile([128, 8], F32, tag="m8")
            nc.vector.max(m8[:], w_sb[:])
            i8 = small.tile([128, 8], U16, tag="i8")
            nc.vector.max_index(i8[:], m8[:], w_sb[:])
            s4 = small.tile([128, 1], F32, tag="s4")
            nc.vector.reduce_sum(s4[:], m8[:, 0:TOPK], axis=mybir.AxisListType.X)
            r4 = small.tile([128, 1], F32, tag="r4")
            nc.vector.reciprocal(r4[:], s4[:])
            nc.vector.tensor_scalar_mul(tw_stage[:, t * TOPK:(t + 1) * TOPK],
                                        m8[:, 0:TOPK], r4[:])
            nc.gpsimd.tensor_copy(te_stage[:, t * TOPK:(t + 1) * TOPK],
                                  i8[:, 0:TOPK])

    # ---- final output transposes + stores ----
    twp = ps_pool.tile([NST * TOPK, 128], F32, tag="ps")
    nc.tensor.transpose(twp[:], tw_stage[:], eye_sb[:])
    tw_sb = small.tile([NST * TOPK, 128], F32, tag="twsb")
    nc.vector.tensor_copy(tw_sb[:], twp[:])
    tep = ps_pool.tile([NST * TOPK, 128], F16, tag="ps")
    nc.tensor.transpose(tep[:], te_stage[:], eyeu_sb[:])
    te_sb = small.tile([NST * TOPK, 128], F16, tag="tesb")
    nc.vector.tensor_copy(te_sb[:], tep[:])

    nc.sync.dma_start(wT[:], wT_stage[:])
    nc.sync.dma_start(twT[:], tw_sb[:])
    nc.sync.dma_start(teT[:], te_sb[:])


_CACHE = {}


def _build_nc():
    if "nc" in _CACHE:
        return _CACHE["nc"]
    nc = bacc.Bacc("TRN2", target_bir_lowering=False, debug=False,
                   num_devices=NCORES)
    ins = {
        "xs": nc.dram_tensor("xs", [TOK, D], F32, kind="ExternalInput").ap(),
        "w128": nc.dram_tensor("w128", [128, KB * E], F32,
                               kind="ExternalInput").ap(),
        "eye": nc.dram_tensor("eye", [128, 128], F32,
                              kind="ExternalInput").ap(),
        "eyeu": nc.dram_tensor("eyeu", [128, 128], F16,
                               kind="ExternalInput").ap(),
        "sum4": nc.dram_tensor("sum4", [128, E], F32,
                               kind="ExternalInput").ap(),
    }
    outs = {
        "wT": nc.dram_tensor("wT", [16, TOK], F32, kind="ExternalOutput").ap(),
        "twT": nc.dram_tensor("twT", [NST * TOPK, 128], F32,
                              kind="ExternalOutput").ap(),
        "teT": nc.dram_tensor("teT", [NST * TOPK, 128], F16,
                              kind="ExternalOutput").ap(),
    }
    with tile.TileContext(nc) as tc:
        with ExitStack() as ctx:
            _router_body(ctx, tc, outs, ins)
    nc.compile()
    _CACHE["nc"] = nc
    return nc


def _prep_w128(W: np.ndarray) -> np.ndarray:
    # W128[32*g + q, 16*k + e] = W[1536*g + 32*k + q, e]
    return np.ascontiguousarray(
        W.reshape(4, KB, 32, E).transpose(0, 2, 1, 3).reshape(128, KB * E)
    )


def _host_inputs(x: np.ndarray, W: np.ndarray):
    xf = np.ascontiguousarray(x.reshape(B * S, D).astype(np.float32, copy=False))
    w128 = _prep_w128(np.asarray(W, dtype=np.float32))
    eye = np.eye(128, dtype=np.float32)
    eyeu = np.eye(128, dtype=np.float16)
    sum4 = np.zeros((128, E), dtype=np.float32)
    for p in range(128):
        if p % 32 < E:
            sum4[p, p % 32] = 1.0
    in_maps = []
    for r in range(NCORES):
        in_maps.append({
            "xs": xf[r * TOK:(r + 1) * TOK],
            "w128": w128,
            "eye": eye,
            "eyeu": eyeu,
            "sum4": sum4,
        })
    return in_maps


def _assemble(results):
    weights = np.empty((B * S, E), dtype=np.float32)
    top_w = np.empty((B * S, TOPK), dtype=np.float32)
    top_e = np.empty((B * S, TOPK), dtype=np.int32)
    for r, res in enumerate(results):
        sl = slice(r * TOK, (r + 1) * TOK)
        weights[sl] = res["wT"].T
        top_w[sl] = (res["twT"].reshape(NST, TOPK, 128)
                     .transpose(0, 2, 1).reshape(TOK, TOPK))
        top_e[sl] = (res["teT"].reshape(NST, TOPK, 128)
                     .transpose(0, 2, 1).reshape(TOK, TOPK)
                     .astype(np.int32))
    return (
        weights.reshape(B, S, E),
        top_w.reshape(B, S, TOPK),
        top_e.reshape(B, S, TOPK),
    )


def run_on_cores(x, W, **run_kwargs):
    """Compile (cached) + run on NeuronCores 0-7; returns BassKernelResults."""
    nc = _build_nc()
    in_maps = _host_inputs(x, W)
    return bass_utils.run_bass_kernel_spmd(
        nc, in_maps, core_ids=list(range(NCORES)), **run_kwargs
    )


def kernel(x: np.ndarray, W: np.ndarray):
    res = run_on_cores(x, W)
    return _assemble(res.results)


# revision 27
# speedup vs baseline: 2.3676x; 2.3676x over previous
"""DBRX router kernel for Trainium2 (8 NeuronCores, data-parallel).

Problem: x [4, 4096, 6144] f32, W [6144, 16] f32
  logits  = x @ W                      (contraction over d=6144)
  weights = softmax(logits, axis=-1)
  top_w, top_e = top_k(weights, 4);  top_w /= sum(top_w)

Design (per core, 2048 tokens):
  * x is loaded in 32-token units: one strided 768 KB DMA per unit places
    (d-block g, token q) on the 128 partitions and d-offset f on the free
    dim; a DVE stream_transpose (32x32 block transpose) then yields tiles
    where partition p holds d = 1536*(p//32) + 32*k + (p%32) and the free
    dim is tokens.  The scrambled d-order is absorbed by pre-permuting W on
    the host, since the matmul contraction is permutation invariant.
  * 48 accumulating K=128 fp32 matmuls per (up to) 256-token group produce
    logits^T in PSUM.  fp32 streams at 1/4 rate, but the M=32 output
    (16 experts zero-padded to 32) uses only one 32-column strip of the PE
    array, so the matmuls are issued 4-way column-tiled
    (tile_position=(0, 32g)): four k-chunks stream concurrently in
    different strips, recovering the 4x.  (float32r would give the same
    rate in one stream, but its ~1e-4 product rounding flips ~20 of 65536
    top-4 expert selections — not exact enough, so exact fp32 it is.)
  * One K=128 select-matmul per 128 tokens (sum4[p, e] = 1 iff p%32 == e)
    folds the four partial strips AND transposes them to token-major
    [128, 16] in a single op.  Softmax is exp (ACT, fused row-sum) *
    reciprocal (DVE); top-4 comes from the DVE max8/max_index
    instructions; top weights are L1-renormalized (the softmax
    normalization cancels, so top_w = top4(exp) / sum4(exp) exactly).
  * All outputs are staged transposed in SBUF so output DMAs are
    contiguous; the host undoes the transposes (outputs are ~1.5 MB
    total).  Expert indices travel as f16 (exact for 0..15) because the
    PE transpose used for the final staging cannot move uint16.
"""

import sys

for _p in ("/opt/trn_rl_repo",):
    if _p not in sys.path:
        sys.path.append(_p)

from contextlib import ExitStack

import numpy as np

import concourse.bacc as bacc
import concourse.mybir as mybir
import concourse.tile as tile
from concourse import bass_utils

# ---- problem constants (hardcoded per contest contract) ----
B, S, D, E = 4, 4096, 6144, 16
TOPK = 4
NCORES = 8
TOK = (B * S) // NCORES          # 2048 tokens per core
NST = TOK // 128                 # 16 x 128-token epilogue tiles
KB = 48                          # matmul K-steps per group (48*128 = 6144)
ME = 32                          # padded matmul M (16 experts + 16 zero cols)

F32 = mybir.dt.float32
U16 = mybir.dt.uint16
F16 = mybir.dt.float16


def _router_body(ctx, tc, outs, ins):
    nc = tc.nc
    xs, w128, eye, eyeu = ins["xs"], ins["w128"], ins["eye"], ins["eyeu"]
    wT, twT, teT = outs["wT"], outs["twT"], outs["teT"]
    Exp = mybir.ActivationFunctionType.Exp
    Copy = mybir.ActivationFunctionType.Copy

    const = ctx.enter_context(tc.tile_pool(name="const", bufs=1))
    xn_pool = ctx.enter_context(tc.tile_pool(name="xn", bufs=6))
    xt_pool = ctx.enter_context(tc.tile_pool(name="xt", bufs=2))
    stage = ctx.enter_context(tc.tile_pool(name="stage", bufs=1))
    small = ctx.enter_context(tc.tile_pool(name="small", bufs=3))
    ps_pool = ctx.enter_context(tc.tile_pool(name="ps", bufs=2, space="PSUM"))
    lp_pool = ctx.enter_context(tc.tile_pool(name="lp", bufs=2, space="PSUM"))
    tp_pool = ctx.enter_context(tc.tile_pool(name="tp", bufs=2, space="PSUM"))

    w128_r = const.tile([128, KB * ME], F32)
    nc.sync.dma_start(w128_r[:], w128[:])
    eye_sb = const.tile([128, 128], F32)
    nc.sync.dma_start(eye_sb[:], eye[:])
    eyeu_sb = const.tile([128, 128], F16)
    nc.sync.dma_start(eyeu_sb[:], eyeu[:])
    sum4_sb = const.tile([128, E], F32)
    nc.sync.dma_start(sum4_sb[:], ins["sum4"][:])

    # Strip-evacuation staging: partitions [32g, 32g+32) carry strip g's
    # partial logits (rows 32g+16.. are computed zeros from the padded W).
    ps_sb = stage.tile([128, 256], F32)
    wT_stage = stage.tile([16, TOK], F32)  # softmax weights^T staging
    tw_stage = stage.tile([128, NST * TOPK], F32)
    te_stage = stage.tile([128, NST * TOPK], F16)

    # DRAM view: unit u covers 32 tokens; element stream in (g, q, f) order,
    # i.e. xs[32*u + q, 1536*g + f].  This matches the SBUF-side tile
    # [128 partitions = (g, q), free = f] element order and keeps both DMA
    # APs within the 3-dim balancer limit, so each 32-token unit is a single
    # 768 KB dma_start.
    xsv = xs.rearrange("(u q) (g f) -> u g q f", q=32, g=4)

    # Group sizes in 32-token units: short first/last groups shrink the
    # pipeline fill (first matmul waits only 4 loads) and drain tail.
    sizes = [4] + [8] * ((TOK // 32 - 8) // 8) + [4]
    u0 = 0
    for gi, ncc in enumerate(sizes):
        # ---- load + on-chip transpose of a group of `ncc` units ----
        xt = xt_pool.tile([128, 8 * 1536], F32, tag="xt")
        for cc in range(ncc):
            u = u0 + cc
            xnu = xn_pool.tile([128, 1536], F32)
            nc.sync.dma_start(xnu[:], xsv[u])
            # After this, xt[p, 1536*cc + 32*k + j] holds
            # x[32*u + j, 1536*(p//32) + 32*k + (p%32)].
            nc.vector.transpose(xt[:, cc * 1536:(cc + 1) * 1536], xnu[:])

        # ---- 48 accumulating matmuls: logits^T, 4-way column-tiled ----
        # fp32 matmuls stream at 1/4 rate, but the output M=32 uses only 32
        # of the PE array's 128 columns.  Column tiling runs 4 k-chunks
        # concurrently in different 32-column strips; strip g accumulates
        # chunks k === g (mod 4) into PSUM partitions [32g, 32(g+1)).
        # (W's M dim is zero-padded 16->32 so strips fill their partitions.)
        ntok = ncc * 32
        ps = ps_pool.tile([128, 256], F32, tag="ps")
        xtr = xt[:].rearrange("p (cc k j) -> p k cc j", cc=8, k=KB, j=32)
        for k in range(KB):
            g = k % 4
            nc.tensor.matmul(
                ps[32 * g:32 * (g + 1), 0:ntok],
                lhsT=w128_r[:, k * ME:(k + 1) * ME],
                rhs=xtr[:, k, 0:ncc],
                start=(k == g),
                stop=(k == KB - 4 + g),
                tile_position=(0, 32 * g),
                # The 4 strips keep independent per-partition accumulation
                # groups inside one PSUM bank; CoreSim's zero-region race
                # bookkeeping can't represent that (numerics are unaffected).
                skip_group_check=True,
            )
        # One evacuation copy; the strips are folded and transposed to
        # token-major by the per-128-token select-matmul below.
        nc.scalar.activation(ps_sb[:, 0:ntok], ps[:, 0:ntok], Copy)

        # ---- per-128-token epilogue ----
        for h in range(ntok // 128):
            t = u0 // 4 + h
            # Fold the 4 strips AND transpose in one matmul: lp[tok, e] =
            # sum_p ps_sb[p, tok] * sum4[p, e], sum4[p, e] = 1 iff p%32 == e.
            lp = lp_pool.tile([128, E], F32)
            nc.tensor.matmul(lp[:], lhsT=ps_sb[:, h * 128:(h + 1) * 128],
                             rhs=sum4_sb[:], start=True, stop=True)
            e_sb = small.tile([128, E], F32, tag="esb")
            s1 = small.tile([128, 1], F32, tag="s1")
            nc.scalar.activation(e_sb[:], lp[:], Exp, accum_out=s1[:])
            r1 = small.tile([128, 1], F32, tag="r1")
            nc.vector.reciprocal(r1[:], s1[:])
            w_sb = small.tile([128, E], F32, tag="wsb")
            nc.vector.tensor_scalar_mul(w_sb[:], e_sb[:], r1[:])
            wtp = tp_pool.tile([16, 128], F32, tag="wtp")
            nc.tensor.transpose(wtp[:], w_sb[:], eye_sb[:])
            nc.scalar.activation(wT_stage[:, t * 128:(t + 1) * 128], wtp[:],
                                 Copy)
            m8 = small.tile([128, 8], F32, tag="m8")
            nc.vector.max(m8[:], w_sb[:])
            i8 = small.tile([128, 8], U16, tag="i8")
            nc.vector.max_index(i8[:], m8[:], w_sb[:])
            s4 = small.tile([128, 1], F32, tag="s4")
            nc.vector.reduce_sum(s4[:], m8[:, 0:TOPK], axis=mybir.AxisListType.X)
            r4 = small.tile([128, 1], F32, tag="r4")
            nc.vector.reciprocal(r4[:], s4[:])
            nc.vector.tensor_scalar_mul(tw_stage[:, t * TOPK:(t + 1) * TOPK],
                                        m8[:, 0:TOPK], r4[:])
            nc.gpsimd.tensor_copy(te_stage[:, t * TOPK:(t + 1) * TOPK],
                                  i8[:, 0:TOPK])
        u0 += ncc

    # ---- final output transposes + stores ----
    twp = ps_pool.tile([NST * TOPK, 128], F32, tag="ps")
    nc.tensor.transpose(twp[:], tw_stage[:], eye_sb[:])
    tw_sb = small.tile([NST * TOPK, 128], F32, tag="twsb")
    nc.vector.tensor_copy(tw_sb[:], twp[:])
    tep = ps_pool.tile([NST * TOPK, 128], F16, tag="ps")
    nc.tensor.transpose(tep[:], te_stage[:], eyeu_sb[:])
    te_sb = small.tile([NST * TOPK, 128], F16, tag="tesb")
    nc.vector.tensor_copy(te_sb[:], tep[:])

    nc.sync.dma_start(wT[:], wT_stage[:])
    nc.sync.dma_start(twT[:], tw_sb[:])
    nc.sync.dma_start(teT[:], te_sb[:])


_CACHE = {}


def _build_nc():
    if "nc" in _CACHE:
        return _CACHE["nc"]
    nc = bacc.Bacc("TRN2", target_bir_lowering=False, debug=False,
                   num_devices=NCORES)
    ins = {
        "xs": nc.dram_tensor("xs", [TOK, D], F32, kind="ExternalInput").ap(),
        "w128": nc.dram_tensor("w128", [128, KB * ME], F32,
                               kind="ExternalInput").ap(),
        "eye": nc.dram_tensor("eye", [128, 128], F32,
                              kind="ExternalInput").ap(),
        "eyeu": nc.dram_tensor("eyeu", [128, 128], F16,
                               kind="ExternalInput").ap(),
        "sum4": nc.dram_tensor("sum4", [128, E], F32,
                               kind="ExternalInput").ap(),
    }
    outs = {
        "wT": nc.dram_tensor("wT", [16, TOK], F32, kind="ExternalOutput").ap(),
        "twT": nc.dram_tensor("twT", [NST * TOPK, 128], F32,
                              kind="ExternalOutput").ap(),
        "teT": nc.dram_tensor("teT", [NST * TOPK, 128], F16,
                              kind="ExternalOutput").ap(),
    }
    with tile.TileContext(nc) as tc:
        with ExitStack() as ctx:
            _router_body(ctx, tc, outs, ins)
    nc.compile()
    _CACHE["nc"] = nc
    return nc


def _prep_w128(W: np.ndarray) -> np.ndarray:
    # W128[32*g + q, 32*k + e] = W[1536*g + 32*k + q, e] for e < 16, else 0.
    # (The M dimension is zero-padded to 32 so each column-tiled strip writes
    # its full 32 PSUM partitions.)
    Wp = np.concatenate([W, np.zeros_like(W)], axis=1)  # [D, 32]
    return np.ascontiguousarray(
        Wp.reshape(4, KB, 32, ME).transpose(0, 2, 1, 3).reshape(128, KB * ME)
    )


def _host_inputs(x: np.ndarray, W: np.ndarray):
    xf = np.ascontiguousarray(x.reshape(B * S, D).astype(np.float32, copy=False))
    w128 = _prep_w128(np.asarray(W, dtype=np.float32))
    eye = np.eye(128, dtype=np.float32)
    eyeu = np.eye(128, dtype=np.float16)
    sum4 = np.zeros((128, E), dtype=np.float32)
    for p in range(128):
        if p % 32 < E:
            sum4[p, p % 32] = 1.0
    in_maps = []
    for r in range(NCORES):
        in_maps.append({
            "xs": xf[r * TOK:(r + 1) * TOK],
            "w128": w128,
            "eye": eye,
            "eyeu": eyeu,
            "sum4": sum4,
        })
    return in_maps


def _assemble(results):
    weights = np.empty((B * S, E), dtype=np.float32)
    top_w = np.empty((B * S, TOPK), dtype=np.float32)
    top_e = np.empty((B * S, TOPK), dtype=np.int32)
    for r, res in enumerate(results):
        sl = slice(r * TOK, (r + 1) * TOK)
        weights[sl] = res["wT"].T
        top_w[sl] = (res["twT"].reshape(NST, TOPK, 128)
                     .transpose(0, 2, 1).reshape(TOK, TOPK))
        top_e[sl] = (res["teT"].reshape(NST, TOPK, 128)
                     .transpose(0, 2, 1).reshape(TOK, TOPK)
                     .astype(np.int32))
    return (
        weights.reshape(B, S, E),
        top_w.reshape(B, S, TOPK),
        top_e.reshape(B, S, TOPK),
    )


def run_on_cores(x, W, **run_kwargs):
    """Compile (cached) + run on NeuronCores 0-7; returns BassKernelResults."""
    nc = _build_nc()
    in_maps = _host_inputs(x, W)
    return bass_utils.run_bass_kernel_spmd(
        nc, in_maps, core_ids=list(range(NCORES)), **run_kwargs
    )


def kernel(x: np.ndarray, W: np.ndarray):
    res = run_on_cores(x, W)
    return _assemble(res.results)
